# revision 35
# baseline (speedup 1.0000x reference)
"""Trainium2 Bass kernel for the CRA relation module (fp16, pair-layout).

Math: the reference computes, per sample,
    phi_x = relu((x@W1+b1)*g1+be1), phi_y likewise,  cat_phi = [phi_x; phi_y]
    A = cat_phi cat_phi^T (symmetric!),  R = [A | A^T] = [A | A]
    W = (cat_phi@W3+b3)@W5a + (R@W4+b4)@W5b + b5
    out = x * W[:196] + y * W[196:]
Because A is symmetric and everything after A is linear into a scalar per
token, the relation pipeline collapses to per-sample matvecs:
    u3 = W3@W5a, u4 = W4@W5b, z = u4[:392]+u4[392:], c0 = b3@W5a+b4@W5b+b5
    s  = u3 + phi_x^T z[:196] + phi_y^T z[196:]          (768-vector)
    W  = phi@s + c0 per token;  out = x*Wx + y*Wy
Data-parallel over batch: 16 samples per core on 8 cores, fp16 on device
(fp32 PSUM accumulation), rel err ~1e-3 vs the 2e-2 gate.

The device computes phi (the dominant 768x768 matmuls), the z-weighted
reductions, and the per-token scalars W; the host applies the final
broadcast scale-and-add out = x*Wx + y*Wy while gathering/unsharding the
8 cores' results (full-precision x,y improve accuracy there).

Device pipeline per group of G=2 samples:
  PE   mains (2x6x6 matmuls of 392 moving rows), per-sample matvec onto
       psum rows 0/32
  ACT  relu psum evictions (pair-major phi [x_a|x_b|y_a|y_b]), W eviction
  DVE  scalar_tensor_tensor z-weighted phi reduction + u3 fold
Weights are host-packed in d-major blocks so the first 196KB DMA unlocks
the d=0 mains; group-0 inputs are k-chunked so compute starts ASAP.
"""

import numpy as np
from contextlib import ExitStack

import concourse.bass as bass
import concourse.tile as tile
import concourse.mybir as mybir
from concourse.bass_utils import run_bass_kernel_spmd

F32 = mybir.dt.float32
F16 = mybir.dt.float16
ALU = mybir.AluOpType
ACTF = mybir.ActivationFunctionType

B, N, C = 128, 196, 768
NCORES = 8
S = B // NCORES          # 16 samples per core
G = 2                    # samples per moving block (392 <= 512 fp32 psum)
NG = S // G              # 8 groups per core
DT = C // 128            # 6 feature tiles
W2T = 2 * N              # 392
PW = 2 * W2T             # 784: phi pair width [x_a|x_b|y_a|y_b]
OW = DT * W2T            # 2352: packed group width


def build_bass(c0: float, for_sim: bool = False) -> bass.Bass:
    nc = bass.Bass()
    xg_d = nc.declare_dram_parameter("xg", [NG, 128, OW], F16, isOutput=False)
    yg_d = nc.declare_dram_parameter("yg", [NG, 128, OW], F16, isOutput=False)
    # d-major weight blocks: w[d] is [128(cin within k), DT*128] with the
    # k-th 128-col block holding W[k*128:(k+1)*128, d*128:(d+1)*128]
    w1_d = nc.declare_dram_parameter("w1", [DT, 128, C], F16, isOutput=False)
    w2_d = nc.declare_dram_parameter("w2", [DT, 128, C], F16, isOutput=False)
    zb_d = nc.declare_dram_parameter("zb", [128, W2T], F16, isOutput=False)
    u3_d = nc.declare_dram_parameter("u3", [128, DT], F32, isOutput=False)
    b1_d = nc.declare_dram_parameter("b1", [128, DT], F32, isOutput=False)
    b2_d = nc.declare_dram_parameter("b2", [128, DT], F32, isOutput=False)
    # per-token scalars: [g, 0] = [Wx_a | Wx_b], [g, 1] = [Wy_a | Wy_b]
    outw_d = nc.declare_dram_parameter("outw", [NG, 2, W2T], F16, isOutput=True)

    with tile.TileContext(nc) as tc, ExitStack() as ctx:
        const = ctx.enter_context(tc.tile_pool(name="const", bufs=1))

        # Biases first (tiny, unlock the ACT warmers). Weight d-blocks and x
        # ride the sync queue; w2/consts ride the scalar (ACT) queue.
        b1t = const.tile([128, DT], F32, tag="b1")
        nc.scalar.dma_start(out=b1t[:], in_=b1_d[:, :])
        b2t = const.tile([128, DT], F32, tag="b2")
        nc.scalar.dma_start(out=b2t[:], in_=b2_d[:, :])
        w1_sb, w2_sb = [], []
        for d in range(DT):
            w1_sb.append(const.tile([128, C], F16, tag=f"w1_{d}",
                                    name=f"w1_{d}"))
            w2_sb.append(const.tile([128, C], F16, tag=f"w2_{d}",
                                    name=f"w2_{d}"))

        def load_weights(ds):
            for d in ds:
                nc.sync.dma_start(out=w1_sb[d][:], in_=w1_d[d])
                nc.scalar.dma_start(out=w2_sb[d][:], in_=w2_d[d])
        # d0 weights split so the first matmul only waits on the k0 slice
        nc.sync.dma_start(out=w1_sb[0][:, 0:128], in_=w1_d[0][:, 0:128])
        nc.scalar.dma_start(out=w2_sb[0][:, 0:128], in_=w2_d[0][:, 0:128])

        zb = const.tile([128, W2T], F16, tag="zb")
        u3 = const.tile([128, DT], F32, tag="u3")
        # Absorb the bias-tile DMA deps into ACT program order now, so the
        # relu evictions later only ever wait on the PE semaphore.
        warm1 = const.tile([128, 1], F32, tag="warm1")
        warm2 = const.tile([128, 1], F32, tag="warm2")
        nc.scalar.activation(warm1[:], b1t[:, 0:1], ACTF.Copy)
        nc.scalar.activation(warm2[:], b2t[:, 0:1], ACTF.Copy)

        xin = ctx.enter_context(tc.tile_pool(name="xin", bufs=3))
        phip = ctx.enter_context(tc.tile_pool(name="phi", bufs=2))
        sp = ctx.enter_context(tc.tile_pool(name="sp", bufs=3))
        ps = ctx.enter_context(tc.tile_pool(name="ps", bufs=2, space="PSUM"))

        zv = zb[:].rearrange("p (a b) -> p a b", a=2)  # [128, 2, 196]

        def emit_mains(g):
            xg = xin.tile([128, OW], F16, tag="xg", name="xg")
            yg = xin.tile([128, OW], F16, tag="yg", name="yg")
            if g == 0:
                # alternate x/y chunks across the two DMA queues so neither
                # stream's k-blocks serialize behind a whole weight tile
                for k in range(DT):
                    kb = slice(k * W2T, (k + 1) * W2T)
                    qx = nc.sync if k % 2 == 0 else nc.scalar
                    qy = nc.scalar if k % 2 == 0 else nc.sync
                    qx.dma_start(out=xg[:, kb], in_=xg_d[g][:, kb])
                    qy.dma_start(out=yg[:, kb], in_=yg_d[g][:, kb])
                    if k == 0:
                        # rest of d0's weights right behind the k0 inputs
                        nc.sync.dma_start(out=w1_sb[0][:, 128:C],
                                          in_=w1_d[0][:, 128:C])
                        nc.scalar.dma_start(out=w2_sb[0][:, 128:C],
                                            in_=w2_d[0][:, 128:C])
                # remaining weights + tail constants land behind group 0's
                # input so the first matmuls aren't queued behind them
                load_weights(range(1, DT))
                nc.scalar.dma_start(out=zb[:], in_=zb_d[:, :])
                nc.scalar.dma_start(out=u3[:], in_=u3_d[:, :])
            else:
                H = OW // 2
                nc.sync.dma_start(out=xg[:, 0:H], in_=xg_d[g][:, 0:H])
                nc.sync.dma_start(out=xg[:, H:OW], in_=xg_d[g][:, H:OW])
                nc.sync.dma_start(out=yg[:, 0:H], in_=yg_d[g][:, 0:H])
                nc.sync.dma_start(out=yg[:, H:OW], in_=yg_d[g][:, H:OW])
            # phi[d]: [128, 784] = [phix_a | phix_b | phiy_a | phiy_b]
            phi = [phip.tile([128, PW], F16, tag=f"phi_{d}", name=f"phi_{d}")
                   for d in range(DT)]
            for d in range(DT):
                psx = ps.tile([128, 512], F32, tag="psx", name="psx", bufs=3)
                psy = ps.tile([128, 512], F32, tag="psy", name="psy", bufs=3)
                for k in range(DT):
                    nc.tensor.matmul(
                        psx[:, 0:W2T], w1_sb[d][:, k * 128:(k + 1) * 128],
                        xg[:, k * W2T:(k + 1) * W2T],
                        start=(k == 0), stop=(k == DT - 1))
                for k in range(DT):
                    nc.tensor.matmul(
                        psy[:, 0:W2T], w2_sb[d][:, k * 128:(k + 1) * 128],
                        yg[:, k * W2T:(k + 1) * W2T],
                        start=(k == 0), stop=(k == DT - 1))
                nc.scalar.activation(phi[d][:, 0:W2T], psx[:, 0:W2T],
                                     ACTF.Relu, bias=b1t[:, d:d + 1])
                nc.scalar.activation(phi[d][:, W2T:PW], psy[:, 0:W2T],
                                     ACTF.Relu, bias=b2t[:, d:d + 1])
            return phi

        def emit_tail(g, phi):
            # s[d] = u3[d] + sum_t z_t phi[d, t] per sample (both streams).
            # The final group goes d-major so each reduction issues as soon
            # as its relu lands (shortens the end-of-kernel drain).
            tsb = [sp.tile([128, DT], F32, tag=f"t_{i}", name=f"t_{i}")
                   for i in range(G)]
            order = ([(i, d) for i in range(G) for d in range(DT)]
                     if g < NG - 1 else
                     [(i, d) for d in range(DT) for i in range(G)])
            for i, d in order:
                scr = sp.tile([128, W2T], F16, tag="scr", name="scr")
                pv = phi[d][:].rearrange("p (a b) -> p a b", a=4)[:, i::2, :]
                sv = scr[:].rearrange("p (a b) -> p a b", a=2)
                nc.vector.scalar_tensor_tensor(
                    out=sv, in0=pv, scalar=1.0, in1=zv,
                    op0=ALU.mult, op1=ALU.mult,
                    accum_out=tsb[i][:, d:d + 1])
            ts = []
            for i in range(G):
                t16 = sp.tile([128, DT], F16, tag=f"t16_{i}", name=f"t16_{i}")
                nc.vector.tensor_tensor(t16[:], tsb[i][:], u3[:], ALU.add)
                ts.append(t16)
            # matvec W = phi^T s: psum row 0 = Wx pair, row 32 = Wy pair
            psw = ps.tile([33, 512], F32, tag="psw", name="psw", bufs=2)
            for i in range(G):
                lo, hi = i * N, (i + 1) * N
                for d in range(DT):
                    nc.tensor.matmul(psw[0:1, lo:hi], ts[i][:, d:d + 1],
                                     phi[d][:, lo:hi],
                                     start=(d == 0), stop=(d == DT - 1))
                for d in range(DT):
                    nc.tensor.matmul(psw[32:33, lo:hi], ts[i][:, d:d + 1],
                                     phi[d][:, W2T + lo:W2T + hi],
                                     start=(d == 0), stop=(d == DT - 1))
            wxy1 = sp.tile([33, W2T], F16, tag="wxy1", name="wxy1")
            nc.scalar.activation(wxy1[0:1, :], psw[0:1, 0:W2T], ACTF.Copy, bias=c0)
            nc.scalar.activation(wxy1[32:33, :], psw[32:33, 0:W2T], ACTF.Copy, bias=c0)
            nc.sync.dma_start(out=outw_d[g, 0], in_=wxy1[0:1, :])
            nc.sync.dma_start(out=outw_d[g, 1], in_=wxy1[32:33, :])

        # Software-pipeline by one group: PE streams mains(g) while group
        # g-1's reduction chain + matvec drain behind it.
        prev = None
        for g in range(NG):
            cur = emit_mains(g)
            if prev is not None:
                emit_tail(g - 1, prev)
            prev = cur
        emit_tail(NG - 1, prev)

    if not for_sim:
        _split_multi_waits(nc)
    return nc


def _split_multi_waits(nc):
    """This walrus build accepts at most ONE sync-wait command per TPB
    instruction; the Tile scheduler happily emits several. Hoist all but the
    last wait of each instruction onto same-engine EventSemaphore ops placed
    immediately before it (engine program order is the within-block
    subsequence, so this preserves semantics)."""
    import json
    data = json.loads(nc.to_json_bytes())
    n = 0
    for fn in data["functions"]:
        for blk in fn["blocks"]:
            out = []
            for inst in blk["instructions"]:
                si = inst.get("sync_info")
                ow = (si or {}).get("on_wait") or []
                if len(ow) > 1:
                    for w in ow[:-1]:
                        n += 1
                        out.append({
                            "name": f"eswait_{n}",
                            "opcode": "EventSemaphore",
                            "engine": inst["engine"],
                            "ins": [],
                            "outs": [],
                            "sync_info": {"on_wait": [w], "on_update": []},
                        })
                    si["on_wait"] = [ow[-1]]
                out.append(inst)
            blk["instructions"] = out
    nc.m = mybir.module_from_json_bytes(json.dumps(data).encode())
    return nc


def pack_weights(W: np.ndarray) -> np.ndarray:
    """[C, C] -> [DT, 128, C] d-major blocks (see w1_d comment)."""
    blk = W.reshape(DT, 128, DT, 128)           # [k, p, d, j]
    return np.ascontiguousarray(
        blk.transpose(2, 1, 0, 3).reshape(DT, 128, C))


def prep_host(inputs: dict):
    x = np.ascontiguousarray(np.asarray(inputs["x"], dtype=np.float32))
    y = np.ascontiguousarray(np.asarray(inputs["y"], dtype=np.float32))
    W1 = np.asarray(inputs["W1"], dtype=np.float32)
    W2 = np.asarray(inputs["W2"], dtype=np.float32)
    g1 = np.asarray(inputs["g1"], dtype=np.float32)
    g2 = np.asarray(inputs["g2"], dtype=np.float32)
    b1 = np.asarray(inputs["b1"], dtype=np.float32)
    b2 = np.asarray(inputs["b2"], dtype=np.float32)
    be1 = np.asarray(inputs["be1"], dtype=np.float32)
    be2 = np.asarray(inputs["be2"], dtype=np.float32)
    W3 = np.asarray(inputs["W3"], dtype=np.float32)
    b3 = np.asarray(inputs["b3"], dtype=np.float32)
    W4 = np.asarray(inputs["W4"], dtype=np.float32)
    b4 = np.asarray(inputs["b4"], dtype=np.float32)
    W5 = np.asarray(inputs["W5"], dtype=np.float32)
    b5 = np.asarray(inputs["b5"], dtype=np.float32)

    W1p = pack_weights(W1 * g1[None, :]).astype(np.float16)
    W2p = pack_weights(W2 * g2[None, :]).astype(np.float16)
    b1p = b1 * g1 + be1
    b2p = b2 * g2 + be2
    W5a, W5b = W5[:C, 0], W5[C:, 0]
    u3 = (W3 @ W5a).astype(np.float32)
    u4 = (W4 @ W5b).astype(np.float32)
    z = (u4[:2 * N] + u4[2 * N:]).astype(np.float32)
    c0 = float(b3 @ W5a + b4 @ W5b + b5[0])

    # [B,N,C] -> per-core groups [M, NG, 128, DT*392] with [x_a|x_b] 392-blocks
    def pack(a):
        at = a.transpose(0, 2, 1).reshape(NCORES, S, DT, 128, N)
        pair = at.reshape(NCORES, NG, G, DT, 128, N)
        gg = np.concatenate([pair[:, :, 0], pair[:, :, 1]], axis=-1)  # [M,NG,DT,128,392]
        return np.ascontiguousarray(
            gg.transpose(0, 1, 3, 2, 4).reshape(NCORES, NG, 128, OW)
            .astype(np.float16))

    XG, YG = pack(x), pack(y)
    zbv = np.ascontiguousarray(
        np.broadcast_to(z[None, :], (128, W2T))).astype(np.float16)
    u3t = np.ascontiguousarray(u3.reshape(DT, 128).T)
    b1t = np.ascontiguousarray(b1p.reshape(DT, 128).T)
    b2t = np.ascontiguousarray(b2p.reshape(DT, 128).T)

    in_maps = []
    for cidx in range(NCORES):
        in_maps.append({
            "xg": XG[cidx], "yg": YG[cidx], "w1": W1p, "w2": W2p,
            "zb": zbv, "u3": u3t, "b1": b1t, "b2": b2t,
        })
    return in_maps, c0, x, y


def gather_w(results) -> tuple[np.ndarray, np.ndarray]:
    """Collect per-token scalars: returns (Wx, Wy) each [B, N] float32."""
    wx = np.empty((B, N), np.float32)
    wy = np.empty((B, N), np.float32)
    for cidx in range(NCORES):
        ow = np.asarray(results[cidx]["outw"]).astype(np.float32)  # [NG,2,392]
        base = cidx * S
        for g in range(NG):
            wx[base + 2 * g] = ow[g, 0, :N]
            wx[base + 2 * g + 1] = ow[g, 0, N:]
            wy[base + 2 * g] = ow[g, 1, :N]
            wy[base + 2 * g + 1] = ow[g, 1, N:]
    return wx, wy


def kernel(**inputs) -> np.ndarray:
    in_maps, c0, x, y = prep_host(inputs)
    nc = build_bass(c0)
    res = run_bass_kernel_spmd(nc, in_maps, list(range(NCORES)))
    wx, wy = gather_w(res.results)
    # final per-token reweighting applied while unsharding (full-precision
    # x, y here improve accuracy over the device-side fp16 copies)
    return x * wx[:, :, None] + y * wy[:, :, None]


# revision 36
# speedup vs baseline: 1.0079x; 1.0079x over previous
"""Trainium2 Bass kernel for the CRA relation module (fp16, pair-layout).

Math: the reference computes, per sample,
    phi_x = relu((x@W1+b1)*g1+be1), phi_y likewise,  cat_phi = [phi_x; phi_y]
    A = cat_phi cat_phi^T (symmetric!),  R = [A | A^T] = [A | A]
    W = (cat_phi@W3+b3)@W5a + (R@W4+b4)@W5b + b5
    out = x * W[:196] + y * W[196:]
Because A is symmetric and everything after A is linear into a scalar per
token, the relation pipeline collapses to per-sample matvecs:
    u3 = W3@W5a, u4 = W4@W5b, z = u4[:392]+u4[392:], c0 = b3@W5a+b4@W5b+b5
    s  = u3 + phi_x^T z[:196] + phi_y^T z[196:]          (768-vector)
    W  = phi@s + c0 per token;  out = x*Wx + y*Wy
Data-parallel over batch: 16 samples per core on 8 cores, fp16 on device
(fp32 PSUM accumulation), rel err ~1e-3 vs the 2e-2 gate.

The device computes phi (the dominant 768x768 matmuls), the z-weighted
reductions, and the per-token scalars W; the host applies the final
broadcast scale-and-add out = x*Wx + y*Wy while gathering/unsharding the
8 cores' results (full-precision x,y improve accuracy there).

Device pipeline per group of G=2 samples:
  PE   mains (2x6x6 matmuls of 392 moving rows), per-sample matvec onto
       psum rows 0/32
  ACT  relu psum evictions (pair-major phi [x_a|x_b|y_a|y_b]), W eviction
  DVE  scalar_tensor_tensor z-weighted phi reduction + u3 fold
Weights are host-packed in d-major blocks so the first 196KB DMA unlocks
the d=0 mains; group-0 inputs are k-chunked so compute starts ASAP.
"""

import numpy as np
from contextlib import ExitStack

import concourse.bass as bass
import concourse.tile as tile
import concourse.mybir as mybir
from concourse.bass_utils import run_bass_kernel_spmd

F32 = mybir.dt.float32
F16 = mybir.dt.float16
ALU = mybir.AluOpType
ACTF = mybir.ActivationFunctionType

B, N, C = 128, 196, 768
NCORES = 8
S = B // NCORES          # 16 samples per core
G = 2                    # samples per moving block (392 <= 512 fp32 psum)
NG = S // G              # 8 groups per core
DT = C // 128            # 6 feature tiles
W2T = 2 * N              # 392
PW = 2 * W2T             # 784: phi pair width [x_a|x_b|y_a|y_b]
OW = DT * W2T            # 2352: packed group width


def build_bass(c0: float, for_sim: bool = False) -> bass.Bass:
    nc = bass.Bass()
    xg_d = nc.declare_dram_parameter("xg", [NG, 128, OW], F16, isOutput=False)
    yg_d = nc.declare_dram_parameter("yg", [NG, 128, OW], F16, isOutput=False)
    # d-major weight blocks: w[d] is [128(cin within k), DT*128] with the
    # k-th 128-col block holding W[k*128:(k+1)*128, d*128:(d+1)*128]
    w1_d = nc.declare_dram_parameter("w1", [DT, 128, C], F16, isOutput=False)
    w2_d = nc.declare_dram_parameter("w2", [DT, 128, C], F16, isOutput=False)
    zb_d = nc.declare_dram_parameter("zb", [128, W2T], F16, isOutput=False)
    u3_d = nc.declare_dram_parameter("u3", [128, DT], F32, isOutput=False)
    b1_d = nc.declare_dram_parameter("b1", [128, DT], F32, isOutput=False)
    b2_d = nc.declare_dram_parameter("b2", [128, DT], F32, isOutput=False)
    # per-token scalars: [g, 0] = [Wx_a | Wx_b], [g, 1] = [Wy_a | Wy_b]
    outw_d = nc.declare_dram_parameter("outw", [NG, 2, W2T], F16, isOutput=True)

    with tile.TileContext(nc) as tc, ExitStack() as ctx:
        const = ctx.enter_context(tc.tile_pool(name="const", bufs=1))

        # Biases first (tiny, unlock the ACT warmers). Weight d-blocks and x
        # ride the sync queue; w2/consts ride the scalar (ACT) queue.
        b1t = const.tile([128, DT], F32, tag="b1")
        nc.scalar.dma_start(out=b1t[:], in_=b1_d[:, :])
        b2t = const.tile([128, DT], F32, tag="b2")
        nc.scalar.dma_start(out=b2t[:], in_=b2_d[:, :])
        w1_sb, w2_sb = [], []
        for d in range(DT):
            w1_sb.append(const.tile([128, C], F16, tag=f"w1_{d}",
                                    name=f"w1_{d}"))
            w2_sb.append(const.tile([128, C], F16, tag=f"w2_{d}",
                                    name=f"w2_{d}"))

        def load_weights(ds):
            for d in ds:
                nc.sync.dma_start(out=w1_sb[d][:], in_=w1_d[d])
                nc.scalar.dma_start(out=w2_sb[d][:], in_=w2_d[d])
        # d0 weights split so the first matmul only waits on the k0 slice
        nc.sync.dma_start(out=w1_sb[0][:, 0:128], in_=w1_d[0][:, 0:128])
        nc.scalar.dma_start(out=w2_sb[0][:, 0:128], in_=w2_d[0][:, 0:128])

        zb = const.tile([128, W2T], F16, tag="zb")
        u3 = const.tile([128, DT], F32, tag="u3")
        # Absorb the bias-tile DMA deps into ACT program order now, so the
        # relu evictions later only ever wait on the PE semaphore.
        warm1 = const.tile([128, 1], F32, tag="warm1")
        warm2 = const.tile([128, 1], F32, tag="warm2")
        nc.scalar.activation(warm1[:], b1t[:, 0:1], ACTF.Copy)
        nc.scalar.activation(warm2[:], b2t[:, 0:1], ACTF.Copy)

        xin = ctx.enter_context(tc.tile_pool(name="xin", bufs=3))
        phip = ctx.enter_context(tc.tile_pool(name="phi", bufs=2))
        sp = ctx.enter_context(tc.tile_pool(name="sp", bufs=3))
        ps = ctx.enter_context(tc.tile_pool(name="ps", bufs=2, space="PSUM"))

        zv = zb[:].rearrange("p (a b) -> p a b", a=2)  # [128, 2, 196]

        def emit_mains(g):
            xg = xin.tile([128, OW], F16, tag="xg", name="xg")
            yg = xin.tile([128, OW], F16, tag="yg", name="yg")
            if g == 0:
                # alternate x/y chunks across the two DMA queues so neither
                # stream's k-blocks serialize behind a whole weight tile
                for k in range(DT):
                    kb = slice(k * W2T, (k + 1) * W2T)
                    qx = nc.sync if k % 2 == 0 else nc.scalar
                    qy = nc.scalar if k % 2 == 0 else nc.sync
                    qx.dma_start(out=xg[:, kb], in_=xg_d[g][:, kb])
                    qy.dma_start(out=yg[:, kb], in_=yg_d[g][:, kb])
                    if k == 0:
                        # rest of d0's weights right behind the k0 inputs
                        nc.sync.dma_start(out=w1_sb[0][:, 128:C],
                                          in_=w1_d[0][:, 128:C])
                        nc.scalar.dma_start(out=w2_sb[0][:, 128:C],
                                            in_=w2_d[0][:, 128:C])
                # remaining weights + tail constants land behind group 0's
                # input so the first matmuls aren't queued behind them
                load_weights(range(1, DT))
                nc.scalar.dma_start(out=zb[:], in_=zb_d[:, :])
                nc.scalar.dma_start(out=u3[:], in_=u3_d[:, :])
            else:
                H = OW // 2
                nc.sync.dma_start(out=xg[:, 0:H], in_=xg_d[g][:, 0:H])
                nc.sync.dma_start(out=xg[:, H:OW], in_=xg_d[g][:, H:OW])
                nc.sync.dma_start(out=yg[:, 0:H], in_=yg_d[g][:, 0:H])
                nc.sync.dma_start(out=yg[:, H:OW], in_=yg_d[g][:, H:OW])
            # phi[d]: [128, 784] = [phix_a | phix_b | phiy_a | phiy_b]
            phi = [phip.tile([128, PW], F16, tag=f"phi_{d}", name=f"phi_{d}")
                   for d in range(DT)]
            for d in range(DT):
                psx = ps.tile([128, 512], F32, tag="psx", name="psx", bufs=3)
                psy = ps.tile([128, 512], F32, tag="psy", name="psy", bufs=3)
                if g == 0 and d == 0:
                    # k-major x/y interleave: each cold-DMA chunk arrival
                    # feeds two matmuls instead of one stream serializing
                    for k in range(DT):
                        nc.tensor.matmul(
                            psx[:, 0:W2T], w1_sb[d][:, k * 128:(k + 1) * 128],
                            xg[:, k * W2T:(k + 1) * W2T],
                            start=(k == 0), stop=(k == DT - 1))
                        nc.tensor.matmul(
                            psy[:, 0:W2T], w2_sb[d][:, k * 128:(k + 1) * 128],
                            yg[:, k * W2T:(k + 1) * W2T],
                            start=(k == 0), stop=(k == DT - 1))
                    continue_relu = True
                else:
                    for k in range(DT):
                        nc.tensor.matmul(
                            psx[:, 0:W2T], w1_sb[d][:, k * 128:(k + 1) * 128],
                            xg[:, k * W2T:(k + 1) * W2T],
                            start=(k == 0), stop=(k == DT - 1))
                    for k in range(DT):
                        nc.tensor.matmul(
                            psy[:, 0:W2T], w2_sb[d][:, k * 128:(k + 1) * 128],
                            yg[:, k * W2T:(k + 1) * W2T],
                            start=(k == 0), stop=(k == DT - 1))
                nc.scalar.activation(phi[d][:, 0:W2T], psx[:, 0:W2T],
                                     ACTF.Relu, bias=b1t[:, d:d + 1])
                nc.scalar.activation(phi[d][:, W2T:PW], psy[:, 0:W2T],
                                     ACTF.Relu, bias=b2t[:, d:d + 1])
            return phi

        def emit_tail(g, phi):
            # s[d] = u3[d] + sum_t z_t phi[d, t] per sample (both streams).
            # The final group goes d-major so each reduction issues as soon
            # as its relu lands (shortens the end-of-kernel drain).
            tsb = [sp.tile([128, DT], F32, tag=f"t_{i}", name=f"t_{i}")
                   for i in range(G)]
            order = ([(i, d) for i in range(G) for d in range(DT)]
                     if g < NG - 1 else
                     [(i, d) for d in range(DT) for i in range(G)])
            for i, d in order:
                scr = sp.tile([128, W2T], F16, tag="scr", name="scr")
                pv = phi[d][:].rearrange("p (a b) -> p a b", a=4)[:, i::2, :]
                sv = scr[:].rearrange("p (a b) -> p a b", a=2)
                nc.vector.scalar_tensor_tensor(
                    out=sv, in0=pv, scalar=1.0, in1=zv,
                    op0=ALU.mult, op1=ALU.mult,
                    accum_out=tsb[i][:, d:d + 1])
            ts = []
            for i in range(G):
                t16 = sp.tile([128, DT], F16, tag=f"t16_{i}", name=f"t16_{i}")
                nc.vector.tensor_tensor(t16[:], tsb[i][:], u3[:], ALU.add)
                ts.append(t16)
            # matvec W = phi^T s: psum row 0 = Wx pair, row 32 = Wy pair
            psw = ps.tile([33, 512], F32, tag="psw", name="psw", bufs=2)
            for i in range(G):
                lo, hi = i * N, (i + 1) * N
                for d in range(DT):
                    nc.tensor.matmul(psw[0:1, lo:hi], ts[i][:, d:d + 1],
                                     phi[d][:, lo:hi],
                                     start=(d == 0), stop=(d == DT - 1))
                for d in range(DT):
                    nc.tensor.matmul(psw[32:33, lo:hi], ts[i][:, d:d + 1],
                                     phi[d][:, W2T + lo:W2T + hi],
                                     start=(d == 0), stop=(d == DT - 1))
            wxy1 = sp.tile([33, W2T], F16, tag="wxy1", name="wxy1")
            nc.scalar.activation(wxy1[0:1, :], psw[0:1, 0:W2T], ACTF.Copy, bias=c0)
            nc.scalar.activation(wxy1[32:33, :], psw[32:33, 0:W2T], ACTF.Copy, bias=c0)
            nc.sync.dma_start(out=outw_d[g, 0], in_=wxy1[0:1, :])
            nc.sync.dma_start(out=outw_d[g, 1], in_=wxy1[32:33, :])

        # Software-pipeline by one group: PE streams mains(g) while group
        # g-1's reduction chain + matvec drain behind it.
        prev = None
        for g in range(NG):
            cur = emit_mains(g)
            if prev is not None:
                emit_tail(g - 1, prev)
            prev = cur
        emit_tail(NG - 1, prev)

    if not for_sim:
        _split_multi_waits(nc)
    return nc


def _split_multi_waits(nc):
    """This walrus build accepts at most ONE sync-wait command per TPB
    instruction; the Tile scheduler happily emits several. Hoist all but the
    last wait of each instruction onto same-engine EventSemaphore ops placed
    immediately before it (engine program order is the within-block
    subsequence, so this preserves semantics)."""
    import json
    data = json.loads(nc.to_json_bytes())
    n = 0
    for fn in data["functions"]:
        for blk in fn["blocks"]:
            out = []
            for inst in blk["instructions"]:
                si = inst.get("sync_info")
                ow = (si or {}).get("on_wait") or []
                if len(ow) > 1:
                    for w in ow[:-1]:
                        n += 1
                        out.append({
                            "name": f"eswait_{n}",
                            "opcode": "EventSemaphore",
                            "engine": inst["engine"],
                            "ins": [],
                            "outs": [],
                            "sync_info": {"on_wait": [w], "on_update": []},
                        })
                    si["on_wait"] = [ow[-1]]
                out.append(inst)
            blk["instructions"] = out
    nc.m = mybir.module_from_json_bytes(json.dumps(data).encode())
    return nc


def pack_weights(W: np.ndarray) -> np.ndarray:
    """[C, C] -> [DT, 128, C] d-major blocks (see w1_d comment)."""
    blk = W.reshape(DT, 128, DT, 128)           # [k, p, d, j]
    return np.ascontiguousarray(
        blk.transpose(2, 1, 0, 3).reshape(DT, 128, C))


def prep_host(inputs: dict):
    x = np.ascontiguousarray(np.asarray(inputs["x"], dtype=np.float32))
    y = np.ascontiguousarray(np.asarray(inputs["y"], dtype=np.float32))
    W1 = np.asarray(inputs["W1"], dtype=np.float32)
    W2 = np.asarray(inputs["W2"], dtype=np.float32)
    g1 = np.asarray(inputs["g1"], dtype=np.float32)
    g2 = np.asarray(inputs["g2"], dtype=np.float32)
    b1 = np.asarray(inputs["b1"], dtype=np.float32)
    b2 = np.asarray(inputs["b2"], dtype=np.float32)
    be1 = np.asarray(inputs["be1"], dtype=np.float32)
    be2 = np.asarray(inputs["be2"], dtype=np.float32)
    W3 = np.asarray(inputs["W3"], dtype=np.float32)
    b3 = np.asarray(inputs["b3"], dtype=np.float32)
    W4 = np.asarray(inputs["W4"], dtype=np.float32)
    b4 = np.asarray(inputs["b4"], dtype=np.float32)
    W5 = np.asarray(inputs["W5"], dtype=np.float32)
    b5 = np.asarray(inputs["b5"], dtype=np.float32)

    W1p = pack_weights(W1 * g1[None, :]).astype(np.float16)
    W2p = pack_weights(W2 * g2[None, :]).astype(np.float16)
    b1p = b1 * g1 + be1
    b2p = b2 * g2 + be2
    W5a, W5b = W5[:C, 0], W5[C:, 0]
    u3 = (W3 @ W5a).astype(np.float32)
    u4 = (W4 @ W5b).astype(np.float32)
    z = (u4[:2 * N] + u4[2 * N:]).astype(np.float32)
    c0 = float(b3 @ W5a + b4 @ W5b + b5[0])

    # [B,N,C] -> per-core groups [M, NG, 128, DT*392] with [x_a|x_b] 392-blocks
    def pack(a):
        at = a.transpose(0, 2, 1).reshape(NCORES, S, DT, 128, N)
        pair = at.reshape(NCORES, NG, G, DT, 128, N)
        gg = np.concatenate([pair[:, :, 0], pair[:, :, 1]], axis=-1)  # [M,NG,DT,128,392]
        return np.ascontiguousarray(
            gg.transpose(0, 1, 3, 2, 4).reshape(NCORES, NG, 128, OW)
            .astype(np.float16))

    XG, YG = pack(x), pack(y)
    zbv = np.ascontiguousarray(
        np.broadcast_to(z[None, :], (128, W2T))).astype(np.float16)
    u3t = np.ascontiguousarray(u3.reshape(DT, 128).T)
    b1t = np.ascontiguousarray(b1p.reshape(DT, 128).T)
    b2t = np.ascontiguousarray(b2p.reshape(DT, 128).T)

    in_maps = []
    for cidx in range(NCORES):
        in_maps.append({
            "xg": XG[cidx], "yg": YG[cidx], "w1": W1p, "w2": W2p,
            "zb": zbv, "u3": u3t, "b1": b1t, "b2": b2t,
        })
    return in_maps, c0, x, y


def gather_w(results) -> tuple[np.ndarray, np.ndarray]:
    """Collect per-token scalars: returns (Wx, Wy) each [B, N] float32."""
    wx = np.empty((B, N), np.float32)
    wy = np.empty((B, N), np.float32)
    for cidx in range(NCORES):
        ow = np.asarray(results[cidx]["outw"]).astype(np.float32)  # [NG,2,392]
        base = cidx * S
        for g in range(NG):
            wx[base + 2 * g] = ow[g, 0, :N]
            wx[base + 2 * g + 1] = ow[g, 0, N:]
            wy[base + 2 * g] = ow[g, 1, :N]
            wy[base + 2 * g + 1] = ow[g, 1, N:]
    return wx, wy


def kernel(**inputs) -> np.ndarray:
    in_maps, c0, x, y = prep_host(inputs)
    nc = build_bass(c0)
    res = run_bass_kernel_spmd(nc, in_maps, list(range(NCORES)))
    wx, wy = gather_w(res.results)
    # final per-token reweighting applied while unsharding (full-precision
    # x, y here improve accuracy over the device-side fp16 copies)
    return x * wx[:, :, None] + y * wy[:, :, None]
